# revision 101
# baseline (speedup 1.0000x reference)
"""CNN-MRF loss (retrieval kNN) on 8 Trainium2 NeuronCores.

Reference: cosine-similarity argmax between all 96x96 content patches and
96x96 style patches (3x3xC=128 patches, d=1152), gather matched style
patches, fold (overlap-add), MSE against content features.

Sharding: content-patch axis N split 8 ways (12 grid rows / core), style
replicated.  Per core, per 128-row content tile:
  coarse: fp8(e4m3) matmuls on host-prenormalized style patch rows
     (scaled x1024) against fp8 content rows -- PSUM directly holds
     cosine scores (content norm is argmax-invariant).  Contraction
     1152 = 4 DoubleRow plane-pairs (pair-interleaved) + 1 plain plane.
     18 m-groups of 512 cols into double-bank PSUM tiles, drained to
     bf16 S rows by the Scalar engine; the first two per-group max
     tournament rounds run per-supergroup right behind the drains.
     fp8 coarse top-1 picks a near-tied neighbor for ~6% of patches;
     MSE rel err ~6e-5 (validated on host).
  argmax: remaining bf16 tensor_tensor max rounds (2x DVE) -> 18 group
     maxes, max8/max_index8 -> winning group; S rows round-trip through
     DRAM so an indirect DMA can fetch each partition's winning 512-wide
     group; max_index8 over 512 -> local index.
  fold: indirect-DMA gather of the matched (un-normalized bf16) style
     patch rows, PE transposes to channel-major (bf16 PSUM), DVE adds
     into one of three rotating bf16 accumulator strips.
Split software pipeline against the in-order engine queues: the argmax
chain for tile j is emitted one matmul round later (all inputs ready, so
it never head-of-line-blocks the DVE), the fold three rounds later (its
transposes' input was gathered two rounds earlier, so the PE never
stalls).  Style chunks stream in chunk-major DRAM layout on the gpsimd
software DGE ahead of the first matmuls.
Host: sums the per-core accumulator strips, divides by fold counts, MSE.
"""
import sys
import numpy as np

for _p in ("/opt/trn_rl_repo",):
    if _p not in sys.path:
        sys.path.insert(0, _p)

import concourse.bass as bass
import concourse.bacc as bacc
import concourse.mybir as mybir
from concourse.bass import IndirectOffsetOnAxis
from concourse.bass_utils import run_bass_kernel_spmd
from concourse.tile import TileContext
from concourse.masks import make_identity

F32 = mybir.dt.float32
BF16 = mybir.dt.bfloat16
F8 = mybir.dt.float8e4
U32 = mybir.dt.uint32

C = 128          # channels
H = W = 96       # feature-map spatial dims
PW = 3           # patch size
HP = H + 2       # padded spatial
N = H * W        # content patches total (9216)
M = N            # style patches (9216)
D = C * PW * PW  # patch vector length (1152)
NCORES = 8
RPC = H // NCORES       # content grid rows per core (12)
NSH = RPC * W           # content patches per core (1152)
NT = NSH // 128         # n-tiles of 128 per core (9)
GW = 512                # m-group width (one PSUM bank of fp32)
NG = M // GW            # m-groups (18)
SG = 3                  # supergroups of 6 groups (3 double-bank tiles)
SCALE = 1024.0          # fp8 quantization scale for normalized style rows


def ts(i, size):
    return slice(i * size, (i + 1) * size)


def build_program():
    nc = bacc.Bacc()

    # chunk-major so each m-chunk is one contiguous DRAM read; planes 0-7
    # pair-interleaved for DoubleRow, plane 8 separate (plain fp8 matmul)
    snorm8 = nc.declare_dram_parameter(
        "snorm8", [NG, C, 4, GW, 2], F8, isOutput=False
    )
    s8_d = nc.declare_dram_parameter("s8_d", [NG, C, GW], F8, isOutput=False)
    cp8 = nc.declare_dram_parameter("cp8", [C, 9, NSH], F8, isOutput=False)
    sprows = nc.declare_dram_parameter("sprows", [M, D], BF16, isOutput=False)
    prow18 = nc.declare_dram_parameter("prow18", [128, 1], U32, isOutput=False)
    idx_out = nc.declare_dram_parameter("idx_out", [NT, 128, 1], U32, isOutput=True)
    racc_out = nc.declare_dram_parameter(
        "racc_out", [3, C, RPC + 2, W], BF16, isOutput=True
    )
    s_dram = nc.dram_tensor("s_scratch", [NT * 128 * NG, GW], BF16)

    DR = mybir.MatmulPerfMode.DoubleRow
    Copy = mybir.ActivationFunctionType.Copy
    MAX = mybir.AluOpType.max

    with TileContext(nc) as tc:
        with (
            tc.tile_pool(name="const", bufs=1) as constp,
            tc.tile_pool(name="big", bufs=1) as bigp,
            tc.tile_pool(name="work", bufs=2) as workp,
            tc.tile_pool(name="psD", bufs=3, space="PSUM") as psD,
            tc.tile_pool(name="psT", bufs=2, space="PSUM") as psT,
        ):
            # style rows first: [c, pair, m, elem] planes 0-7, plane 8 separate
            snorm_t = bigp.tile([C, 4, M, 2], F8)
            s8_t = bigp.tile([C, M], F8)
            for g in range(NG):
                nc.gpsimd.dma_start(out=snorm_t[:, :, ts(g, GW), :], in_=snorm8[g])
            cp_t = bigp.tile([C, 9, NSH], F8)
            nc.sync.dma_start(out=cp_t[:, 0:5], in_=cp8[:, 0:5])
            nc.scalar.dma_start(out=cp_t[:, 5:9], in_=cp8[:, 5:9])
            for g in range(NG):
                nc.sync.dma_start(out=s8_t[:, ts(g, GW)], in_=s8_d[g])
            prow_t = constp.tile([128, 1], U32)
            nc.sync.dma_start(out=prow_t[:], in_=prow18[:])

            ident = constp.tile([128, 128], BF16)
            make_identity(nc, ident[:])
            # three fold accumulators so consecutive tiles' adds don't chain
            raccs = []
            for h in range(3):
                racc_h = bigp.tile([C, RPC + 2, HP], BF16, name=f"racc{h}")
                nc.gpsimd.memset(racc_h[:], 0.0)
                raccs.append(racc_h)

            def emit_mm(j):
                """Coarse fp8 matmuls + scalar drains + S->DRAM dump."""
                S_sb = bigp.tile(
                    [C, NG * GW], BF16, tag="S_sb", bufs=3, name=f"S_{j}"
                )
                tmax = workp.tile(
                    [128, NG, 256], BF16, tag="tmax", bufs=3, name=f"tm_{j}"
                )
                S3 = S_sb[:].rearrange("p (a b) -> p a b", b=GW)
                for sg in range(SG):
                    pds = [
                        psD.tile([128, 2 * GW], F32, tag="psD", name=f"pd_{j}_{sg}_{t}")
                        for t in range(3)
                    ]
                    for kp in range(5):
                        if kp < 4:
                            lhsT = cp_t[:, 2 * kp : 2 * kp + 2, ts(j, 128)]
                        else:
                            lhsT = cp_t[:, 8, ts(j, 128)]
                        for t in range(3):
                            for h in range(2):
                                g = sg * 6 + t * 2 + h
                                if kp < 4:
                                    nc.tensor.matmul(
                                        out=pds[t][:, ts(h, GW)],
                                        lhsT=lhsT,
                                        rhs=snorm_t[:, kp, ts(g, GW), :].rearrange(
                                            "p n e -> p e n"
                                        ),
                                        start=(kp == 0),
                                        stop=False,
                                        perf_mode=DR,
                                    )
                                else:
                                    nc.tensor.matmul(
                                        out=pds[t][:, ts(h, GW)],
                                        lhsT=lhsT,
                                        rhs=s8_t[:, ts(g, GW)],
                                        start=False,
                                        stop=True,
                                    )
                    for t in range(3):
                        g0 = sg * 6 + t * 2
                        nc.scalar.activation(
                            S_sb[:, g0 * GW : (g0 + 2) * GW], pds[t][:], Copy
                        )
                    # dump this supergroup's rows for the later indirect fetch
                    nc.sync.dma_start(
                        out=s_dram[:]
                        .rearrange("(a p g) w -> a p g w", a=NT, p=128)[j][
                            :, ts(sg, 6), :
                        ]
                        .rearrange("p g w -> p (g w)"),
                        in_=S_sb[:, sg * 6 * GW : (sg + 1) * 6 * GW],
                    )
                    # first two tournament rounds on this supergroup's rows
                    gs = ts(sg, 6)
                    nc.vector.tensor_tensor(
                        out=tmax[:, gs, :],
                        in0=S3[:, gs, 0:256],
                        in1=S3[:, gs, 256:512],
                        op=MAX,
                    )
                    nc.vector.tensor_tensor(
                        out=tmax[:, gs, 0:128],
                        in0=tmax[:, gs, 0:128],
                        in1=tmax[:, gs, 128:256],
                        op=MAX,
                    )
                return (S_sb, tmax)

            def emit_argmax(j, S_sb, tmax):
                """Argmax + gathers for tile j (emitted at depth 1)."""
                w = 64
                while w >= 8:
                    nc.vector.tensor_tensor(
                        out=tmax[:, :, 0:w],
                        in0=tmax[:, :, 0:w],
                        in1=tmax[:, :, w : 2 * w],
                        op=MAX,
                    )
                    w //= 2
                gmax = workp.tile([128, NG], BF16, tag="gmax", name=f"gm_{j}")
                nc.vector.tensor_reduce(
                    gmax[:], tmax[:, :, 0:8], axis=mybir.AxisListType.X, op=MAX
                )
                gmax8 = workp.tile([128, 8], BF16, tag="gmax8", name=f"gm8_{j}")
                nc.vector.max(gmax8[:], gmax[:])
                gstar8 = workp.tile([128, 8], U32, tag="gstar8", name=f"gs8_{j}")
                nc.vector.max_index(gstar8[:], gmax8[:], gmax[:])

                rowid = workp.tile([128, 1], U32, tag="rowid", name=f"ri_{j}")
                nc.vector.tensor_tensor(
                    out=rowid[:], in0=prow_t[:], in1=gstar8[:, 0:1],
                    op=mybir.AluOpType.add,
                )
                rowid2 = workp.tile([128, 1], U32, tag="rowid2", name=f"ri2_{j}")
                nc.vector.tensor_scalar(
                    out=rowid2[:], in0=rowid[:], scalar1=j * 128 * NG, scalar2=None,
                    op0=mybir.AluOpType.add,
                )
                wrow = workp.tile([128, GW], BF16, tag="wrow", name=f"wr_{j}")
                nc.gpsimd.indirect_dma_start(
                    out=wrow[:],
                    out_offset=None,
                    in_=s_dram[:],
                    in_offset=IndirectOffsetOnAxis(ap=rowid2[:, 0:1], axis=0),
                )
                li8 = workp.tile([128, 8], U32, tag="li8", name=f"li_{j}")
                nc.vector.max_index(li8[:], gmax8[:], wrow[:])

                g512 = workp.tile([128, 1], U32, tag="g512", name=f"g5_{j}")
                nc.vector.tensor_scalar(
                    out=g512[:], in0=gstar8[:, 0:1], scalar1=GW, scalar2=None,
                    op0=mybir.AluOpType.mult,
                )
                bestu = workp.tile([128, 1], U32, tag="bestu", name=f"bu_{j}")
                nc.vector.tensor_tensor(
                    out=bestu[:], in0=g512[:], in1=li8[:, 0:1],
                    op=mybir.AluOpType.add,
                )
                nc.sync.dma_start(out=idx_out[j], in_=bestu[:])

                # gather matched (un-normalized) style patch rows (n-major)
                matched = workp.tile(
                    [128, D], BF16, tag="matched", bufs=4, name=f"ma_{j}"
                )
                nc.gpsimd.indirect_dma_start(
                    out=matched[:],
                    out_offset=None,
                    in_=sprows[:],
                    in_offset=IndirectOffsetOnAxis(ap=bestu[:, 0:1], axis=0),
                )
                return (matched,)

            def emit_fold(j, matched, pools=None):
                """Transpose + fold-accumulate for tile j (emitted at depth 3)."""
                if pools is None:
                    pools = (psT,)
                matched3 = matched[:].rearrange("p (a b) -> p a b", b=9)
                racc = raccs[j % 3]

                # transpose to channel-major and fold-accumulate
                n0 = j * 128
                r0, c0 = n0 // W, n0 % W
                seg1 = (r0, c0, W - c0, 0)
                seg2 = (r0 + 1, 0, 128 - (W - c0), W - c0)
                for k in range(9):
                    ki, kj = k // 3, k % 3
                    pool_k = pools[k % len(pools)]
                    psum_T = pool_k.tile(
                        [128, 128], BF16, tag="psT", name=f"pT_{j}_{k}"
                    )
                    sl = psum_T[:]
                    nc.tensor.transpose(sl, matched3[:, :, k], ident[:])
                    # deprioritize the racc adds: the scheduler's matmul cost
                    # model runs ~2x fast vs HW, so without this it queues
                    # them too early in the in-order DVE stream
                    old_pri = tc.cur_priority
                    tc.cur_priority = old_pri + 600
                    for (r, c, ln, off) in (seg1, seg2):
                        nc.vector.tensor_add(
                            racc[:, r + ki, c + kj : c + kj + ln],
                            racc[:, r + ki, c + kj : c + kj + ln],
                            sl[:, off : off + ln],
                        )
                    tc.cur_priority = old_pri

            # split software pipeline: the argmax chain runs at depth 1 (all
            # inputs ready the moment it's emitted), the fold at depth 3 (its
            # transposes' input was gathered two rounds earlier)
            mm_out = {}
            am_out = {}
            for j in range(NT):
                mm_out[j] = emit_mm(j)
                if j >= 1:
                    am_out[j - 1] = emit_argmax(j - 1, *mm_out.pop(j - 1))
                if j >= 3:
                    emit_fold(j - 3, *am_out.pop(j - 3))
            def emit_fold_pair(ja, ma, jb, mb):
                """Interleave two ready folds k-wise across their raccs."""
                specs = []
                for j, m in ((ja, ma), (jb, mb)):
                    m3 = m[:].rearrange("p (a b) -> p a b", b=9)
                    n0 = j * 128
                    r0, c0 = n0 // W, n0 % W
                    segs = (
                        (r0, c0, W - c0, 0),
                        (r0 + 1, 0, 128 - (W - c0), W - c0),
                    )
                    specs.append((j, m3, raccs[j % 3], segs))
                for k in range(9):
                    ki, kj = k // 3, k % 3
                    for (j, m3, racc, segs) in specs:
                        psum_T = psT.tile(
                            [128, 128], BF16, tag="psT", name=f"pTf_{j}_{k}"
                        )
                        nc.tensor.transpose(psum_T[:], m3[:, :, k], ident[:])
                        old_pri = tc.cur_priority
                        tc.cur_priority = old_pri + 600
                        for (r, c, ln, off) in segs:
                            nc.vector.tensor_add(
                                racc[:, r + ki, c + kj : c + kj + ln],
                                racc[:, r + ki, c + kj : c + kj + ln],
                                psum_T[:, off : off + ln],
                            )
                        tc.cur_priority = old_pri

            am_out[NT - 1] = emit_argmax(NT - 1, *mm_out.pop(NT - 1))
            emit_fold_pair(
                NT - 3, am_out.pop(NT - 3)[0], NT - 2, am_out.pop(NT - 2)[0]
            )
            nc.sync.dma_start(out=racc_out[0], in_=raccs[0][:, :, 1 : 1 + W])
            nc.scalar.dma_start(out=racc_out[1], in_=raccs[1][:, :, 1 : 1 + W])
            emit_fold(NT - 1, *am_out.pop(NT - 1))
            nc.sync.dma_start(out=racc_out[2], in_=raccs[2][:, :, 1 : 1 + W])

    if not nc.is_finalized():
        nc.finalize()
    return nc


_PROGRAM = None


def _get_program():
    global _PROGRAM
    if _PROGRAM is None:
        _PROGRAM = build_program()
    return _PROGRAM


def _patch_rows(x):
    """(C, R, Cc) padded map -> ((R-2)*(Cc-2), C*9) patch rows, (c,ki,kj)."""
    w = np.lib.stride_tricks.sliding_window_view(x, (PW, PW), axis=(1, 2))
    return np.ascontiguousarray(
        w.transpose(1, 2, 0, 3, 4).reshape((x.shape[1] - 2) * (x.shape[2] - 2), -1)
    )


def _pair_interleave(rows_T, n):
    """(1152, n) f32 plane-major -> planes 0-7 as (128, 4, n, 2), plane 8."""
    a = rows_T.reshape(C, PW * PW, n)
    pairs = np.ascontiguousarray(a[:, 0:8].reshape(C, 4, 2, n).transpose(0, 1, 3, 2))
    return pairs, np.ascontiguousarray(a[:, 8])


def _host_prep(content_feats, style_feats):
    """Build per-core input maps."""
    f8 = mybir.dt.np(F8)
    cf = np.ascontiguousarray(np.asarray(content_feats, dtype=np.float32)[0])
    sf = np.ascontiguousarray(np.asarray(style_feats, dtype=np.float32)[0])
    cpad = np.pad(cf, ((0, 0), (1, 1), (1, 1)))
    spad = np.pad(sf, ((0, 0), (1, 1), (1, 1)))
    bf = mybir.dt.np(BF16)
    sprows = _patch_rows(spad)
    invn = 1.0 / np.maximum(
        np.linalg.norm(sprows, axis=1), np.float32(1e-12)
    ).astype(np.float32)
    snormq = (
        (sprows * (np.float32(SCALE) * invn)[:, None]).astype(f8).astype(np.float32)
    )
    pairs, s8p = _pair_interleave(np.ascontiguousarray(snormq.T), M)
    snorm8 = np.ascontiguousarray(
        pairs.reshape(C, 4, NG, GW, 2).transpose(2, 0, 1, 3, 4)
    ).astype(f8)
    s8_d = np.ascontiguousarray(
        s8p.reshape(C, NG, GW).transpose(1, 0, 2)
    ).astype(f8)
    # quantize the content map once; rows are shifted views of the map
    cpadq = cpad.astype(f8).astype(np.float32)
    prow = (np.arange(128, dtype=np.uint32) * NG).reshape(128, 1)
    in_maps = []
    for i in range(NCORES):
        cslab = np.ascontiguousarray(cpadq[:, i * RPC : i * RPC + RPC + 2, :])
        crows = _patch_rows(cslab)                      # (NSH, 1152)
        cp8 = (
            np.ascontiguousarray(crows.T).reshape(C, PW * PW, NSH).astype(f8)
        )
        in_maps.append(
            {
                "snorm8": snorm8,
                "s8_d": s8_d,
                "cp8": cp8,
                "sprows": sprows.astype(bf),
                "prow18": prow,
            }
        )
    return cf, in_maps


_DIVISOR = None


def _fold_divisor():
    global _DIVISOR
    if _DIVISOR is None:
        cnt = np.full(H, 3, dtype=np.float32)
        cnt[0] = cnt[-1] = 2
        _DIVISOR = np.outer(cnt, cnt).astype(np.float32) + np.float32(1e-8)
    return _DIVISOR


def _host_combine(cf, results):
    acc = np.zeros((C, H + 2, W), dtype=np.float32)
    for i in range(NCORES):
        ro = results[i]["racc_out"].astype(np.float32)
        acc[:, i * RPC : i * RPC + RPC + 2, :] += ro.sum(axis=0)
    recon = acc[:, 1 : 1 + H, :] / _fold_divisor()[None, :, :]
    diff = cf - recon
    return np.float32(np.mean(np.square(diff), dtype=np.float64))


def run(content_feats, style_feats, trace=False):
    nc = _get_program()
    cf, in_maps = _host_prep(content_feats, style_feats)
    res = run_bass_kernel_spmd(
        nc, in_maps, core_ids=list(range(NCORES)), trace=trace
    )
    mse = _host_combine(cf, res.results)
    return mse, res


def kernel(content_feats, style_feats):
    mse, _ = run(content_feats, style_feats)
    return np.array(mse, dtype=np.float32)
